# revision 65
# baseline (speedup 1.0000x reference)
"""AttentionReadout kernel for Trainium2 (8 NeuronCores, data-parallel by chunk).

Reference computation (per full input):
    scores = (tanh(x @ W1 + b1) @ W2)[:, 0]          # [N]
    chunk_id = batch // 32                            # 32 graphs per chunk
    w = softmax of scores within each chunk           # [N]
    out = segment_sum(w[:, None] * x, batch)          # [4096, 256]

Shapes: x [262144, 256] f32, batch [262144] i64 (sorted, uniform: 64
nodes/graph), W1 [256,256], b1 [256], W2 [256,1].

Strategy (per core, 32768 nodes = 16 chunks of 2048 nodes):
  - host: ship x twice: natural layout in bf16 (pooling rhs; pooling
    needs ~bf16 precision to clear the 2e-2 gate) and transposed layout
    in fp8-e4m3 (MLP rhs; fp8 halves that DMA stream and enables
    DoubleRow matmuls). W1 is scaled by 64 (avoids the e4m3 subnormal
    region) and shipped as an e4m3 hi/lo pair so its quantization error
    cancels to ~0.1%; measured end-to-end rel err 8.3e-3 on the
    reference seed vs 2.5e-3 for all-bf16.
  - device, per chunk:
      hT = W1.T @ xT          (PE, fp8 DoubleRow: K=256 per matmul via
                               paired k-tiles; hi+lo accumulate in psum)
      th = tanh(hT/64 + b1)   (ACT, psum->sbuf bf16, scale=1/64)
      s[n] = th.T @ W2        (PE, tanh tile as stationary operand -> s in
                               node-partition layout [128,16] psum)
      e = exp(s), rowsum      (ACT fused accum_out)
      D = allreduce(rowsum)   (GPSIMD partition_all_reduce)
      w = e * (1/D)           (DVE)
      E[n, g] = w * mask      (DVE, mask precomputed on host)
      out[g,:] = E.T @ x      (PE, accumulate 16 node-tiles per chunk)
  - softmax max-subtraction is skipped: scores = tanh(.)@W2 are bounded by
    sum|W2| <= 16, so exp() cannot overflow in f32 and w = e/sum(e) is
    mathematically identical to the max-shifted form.
"""

import numpy as np
import ml_dtypes

import concourse.bass as bass
import concourse.bacc as bacc
import concourse.tile as tile
import concourse.mybir as mybir
import concourse.bass_isa as bass_isa
from concourse.bass_utils import run_bass_kernel_spmd

BF16 = mybir.dt.bfloat16
F32 = mybir.dt.float32
FP8 = mybir.dt.float8e4
NP_BF16 = ml_dtypes.bfloat16
NP_E4M3 = ml_dtypes.float8_e4m3
W1_SCALE = 64.0  # shifts W1 (~±1/16) out of e4m3's subnormal range

N_CORES = 8
HIDDEN = 256
CHUNK_GRAPHS = 32
GRAPH_NODES = 64          # uniform: nodes per graph
TILE_NODES = 128          # nodes per node-tile (SBUF partition dim)
CHUNK_NODES = CHUNK_GRAPHS * GRAPH_NODES      # 2048
TILES_PER_CHUNK = CHUNK_NODES // TILE_NODES   # 16
BLOCKS_PER_CHUNK = 4                          # sub-blocks of 512 nodes
BLOCK_NODES = 512

_NC_CACHE = {}

HPSUM_BUFS = 3
# build-bisection switches (sim experiments only; all True for real runs)
EN_XNAT = True     # load x_nat
EN_POOL = True     # pooling matmuls + store
EN_SCORE = True    # score matmuls + softmax chain
EN_TANH = True     # tanh activations
POOL_FAKE_E = False  # probe: pool with mask (always ready) instead of E


def build_nc(n_chunks, repeat=1, out_name="out", salt=0, score_fmajor=False):
    """Build the per-core Bass program (identical across cores)."""
    nc = bacc.Bacc("TRN2", target_bir_lowering=False, debug=False,
                   enable_asserts=False)

    nodes = n_chunks * CHUNK_NODES
    # DRAM I/O (per-core shard)
    x_nat_d = nc.dram_tensor(
        "x_nat", [n_chunks, TILE_NODES, TILES_PER_CHUNK, HIDDEN], BF16,
        kind="ExternalInput").ap()
    x_tr_d = nc.dram_tensor(
        "x_tr", [2, 128, nodes], FP8, kind="ExternalInput").ap()
    # [p, hl, kt, mt, j]: e4m3 hi/lo pair of W1*64
    w1_d = nc.dram_tensor("w1", [128, 2, 2, 2, 128], FP8,
                          kind="ExternalInput").ap()
    if score_fmajor:
        w2_d = nc.dram_tensor("w2r", [128, 2, 128], BF16,
                              kind="ExternalInput").ap()
    else:
        w2_d = nc.dram_tensor("w2", [128, 2], BF16, kind="ExternalInput").ap()
    b1_d = nc.dram_tensor("b1", [128, 2], F32, kind="ExternalInput").ap()
    mask_d = nc.dram_tensor(
        "maskw", [TILE_NODES, CHUNK_GRAPHS, TILES_PER_CHUNK], BF16,
        kind="ExternalInput").ap()
    out_d = nc.dram_tensor(
        out_name, [n_chunks * CHUNK_GRAPHS, HIDDEN], F32,
        kind="ExternalOutput").ap()
    if salt:
        # dummy input whose shape varies per build variant: defeats
        # executable-cache dedup between otherwise identical HLO programs
        nc.dram_tensor("salt", [1, salt], F32, kind="ExternalInput")

    with tile.TileContext(nc) as tc:
        with (
            tc.tile_pool(name="consts", bufs=1) as consts,
            tc.tile_pool(name="xpool", bufs=6) as xpool,
            tc.tile_pool(name="xtpool", bufs=5) as xtpool,
            tc.tile_pool(name="thpool", bufs=10) as thpool,
            tc.tile_pool(name="epool", bufs=3) as epool,
            tc.tile_pool(name="opool", bufs=3) as opool,
            tc.tile_pool(name="swpool", bufs=2) as swpool,
            tc.tile_pool(name="hpsum", bufs=HPSUM_BUFS, space="PSUM") as hpsum,
            tc.tile_pool(name="spsum", bufs=1, space="PSUM") as spsum,
            tc.tile_pool(name="ppsum", bufs=1, space="PSUM") as ppsum,
        ):
            # The ACT function-table load (1.3us) starts at t~0 off a dummy
            # tanh whose source is a DVE-memset tile — no DMA dependency.
            warm_sb = consts.tile([128, 2], F32, name="act_warm")
            nc.vector.memset(warm_sb[:, 0:1], 0.0)
            nc.scalar.activation(warm_sb[:, 1:2], warm_sb[:, 0:1],
                                 mybir.ActivationFunctionType.Tanh)
            # DMA issue order is the fill critical path: the SP sequencer
            # spends ~565ns per dma_start, so the MLP inputs (w1, b1, then
            # xt(0) interleaved below) go first.
            w1_sb = consts.tile([128, 2, 2, 2, 128], FP8)
            nc.sync.dma_start(out=w1_sb, in_=w1_d)
            b1_sb = consts.tile([128, 2], F32)
            nc.sync.dma_start(out=b1_sb, in_=b1_d)

            # Persistent psum accumulators, two logical buffers packed into a
            # single bank each (the score tile is 64B/partition and the pool
            # tile 1KB/partition, but the tile-pool allocator rounds each buf
            # up to a full 2KB bank). Alternating chunks use the two halves;
            # a start=True matmul pending-zeroes its whole bank, so the
            # framework orders each chunk's first score/pool matmul after the
            # other half's last reader (exp/copy of chunk c-1) — about half
            # an iteration of slack. This frees two banks, buying the MLP
            # psum a third buffer (decouples the PE->ACT tanh ping-pong).
            s_both = spsum.tile([128, 2, TILES_PER_CHUNK], F32, name="s_both")
            p_both = ppsum.tile([CHUNK_GRAPHS, 2, HIDDEN], F32, name="p_both")

            # Software pipeline, 3 chunks deep on the PE:
            #   iteration c emits, round-robin per node-tile:
            #     MLP matmuls of chunk c   (F=512 streams)
            #     score matmuls of c-1     (weight-load bound; hides under MLP)
            #     pooling matmuls of c-2   (F=256 streams)
            #   plus tanh(c) on ACT and the softmax chain (c-1) on
            #   ACT/GPSIMD/DVE, which runs while chunk c streams.
            st = {}  # per-chunk live tiles  (reassigned per repeat)

            def emit_load_xt(c, split=1):
                # xt issues at the top of the iteration: the HWDGE ring is
                # FIFO, and the transposed layout feeds the MLP immediately,
                # while the natural layout is only needed two pipeline
                # iterations later (pooling) and is queued at the bottom.
                # For chunk 0 the load is split so the first MLP block can
                # start after ~1/4 of the transfer (shorter pipeline fill).
                xt_sb = xtpool.tile([128, 2, CHUNK_NODES], FP8, tag="xt")
                step = CHUNK_NODES // split
                for s in range(split):
                    lo = s * step
                    nc.sync.dma_start(
                        out=xt_sb[:, :, lo:lo + step],
                        in_=x_tr_d[:, :, c * CHUNK_NODES + lo:
                                   c * CHUNK_NODES + lo + step]
                        .transpose([1, 0, 2]))
                st[c] = {"xt": xt_sb, "th": {}}

            def emit_load_xnat(c):
                x_sb = xpool.tile([TILE_NODES, TILES_PER_CHUNK, HIDDEN], BF16,
                                  tag="x")
                nc.sync.dma_start(out=x_sb, in_=x_nat_d[c])
                st[c]["x"] = x_sb

            def mlp_tiles(total_tiles):
                """Global MLP tile stream; yields after each DR matmul
                (4 per tile: 2 node-blocks x W1 hi/lo)."""
                for g in range(total_tiles):
                    c, r = divmod(g, 4)
                    bp, mt = divmod(r, 2)
                    xt_sb = st[c]["xt"]
                    h_ps = hpsum.tile([128, 2, BLOCK_NODES], F32, tag="h",
                                      name=f"h_ps{c}_{bp}_{mt}")
                    for bb in range(2):
                        nlo = (2 * bp + bb) * BLOCK_NODES
                        # DoubleRow: lhsT [128, kt=2, 128] pairs with
                        # rhs [128, kt=2, 512] -> full K=256 per matmul;
                        # hi then lo of W1*64 accumulate into psum.
                        for hl in range(2):
                            with tc.high_priority():
                                nc.tensor.matmul(
                                    h_ps[:, bb, :], w1_sb[:, hl, :, mt, :],
                                    xt_sb[:, :, nlo:nlo + BLOCK_NODES],
                                    start=(hl == 0), stop=(hl == 1),
                                    perf_mode=mybir.MatmulPerfMode.DoubleRow)
                            yield
                    th = thpool.tile([128, 2, BLOCK_NODES], BF16, tag="th",
                                     name=f"th{c}_{bp}_{mt}")
                    if EN_TANH:
                        with tc.high_priority():
                            nc.scalar.activation(
                                th, h_ps,
                                mybir.ActivationFunctionType.Tanh,
                                bias=b1_sb[:, mt:mt + 1],
                                scale=1.0 / W1_SCALE)
                    st[c]["th"][(bp, mt)] = th

            def score_ops(c):
                """8 slots; slot si emits the 4 accumulating F=1 matmuls for
                node-tiles 2si and 2si+1 (tanh as stationary operand).
                Compressed into the first half of the iteration so the exp
                for this chunk can issue mid-iteration, filling the ACT gap
                between tanh tiles and shortening the softmax chain."""
                ops = []
                for si in range(TILES_PER_CHUNK // 4):
                    def op(si=si, c=c):
                        s_ps = st[c]["s"]
                        for t in range(4 * si, 4 * si + 4):
                            b, tl = divmod(t, 4)
                            bp, bb = divmod(b, 2)
                            for mt in range(2):
                                th = st[c]["th"][(bp, mt)]
                                nc.tensor.matmul(
                                    s_ps[:, t:t + 1],
                                    th[:, bb, tl * 128:(tl + 1) * 128],
                                    w2_sb[:, mt:mt + 1],
                                    start=(mt == 0), stop=(mt == 1))
                    ops.append(op)
                return ops

            def emit_softmax(c):
                # Pool with UNNORMALIZED weights E = exp(s)*mask; the 1/D
                # factor is applied per-graph on the output copy instead.
                # This takes the accumulator read off ACT and the reciprocal
                # off the exp->pool critical chain (it overlaps pooling).
                e_sb = epool.tile([128, TILES_PER_CHUNK], BF16, tag="e")
                nc.scalar.activation(
                    e_sb, st[c]["s"], mybir.ActivationFunctionType.Exp)
                e_full = epool.tile(
                    [TILE_NODES, CHUNK_GRAPHS, TILES_PER_CHUNK], BF16,
                    tag="efull")
                e_bc = e_sb.unsqueeze(1).broadcast_to(
                    [TILE_NODES, CHUNK_GRAPHS, TILES_PER_CHUNK])
                nc.vector.tensor_mul(e_full, e_bc, mask_sb)
                acc = epool.tile([128, 1], F32, tag="acc")
                nc.vector.tensor_reduce(
                    acc, e_sb, mybir.AxisListType.X, mybir.AluOpType.add)
                dsum = epool.tile([128, 1], F32, tag="dsum")
                nc.gpsimd.partition_all_reduce(
                    dsum, acc, 128, bass_isa.ReduceOp.add)
                rden = epool.tile([128, 1], F32, tag="rden")
                nc.vector.reciprocal(rden, dsum)
                st[c]["rden"] = rden
                st[c]["E"] = e_full
                st[c]["p"] = p_both[:, c % 2, :]

            def pool_ops(c):
                ops = []
                for t in range(TILES_PER_CHUNK):
                    def op(t=t, c=c):
                        e_op = mask_sb if POOL_FAKE_E else st[c]["E"]
                        nc.tensor.matmul(
                            st[c]["p"], e_op[:, :, t], st[c]["x"][:, t, :],
                            start=(t == 0), stop=(t == TILES_PER_CHUNK - 1))
                    ops.append(op)
                return ops

            def emit_copy_out(c):
                o_sb = opool.tile([CHUNK_GRAPHS, HIDDEN], F32, tag="o")
                # normalize while copying: all partitions of rden hold the
                # chunk's all-reduced 1/D, so rows 0-31 see the same scalar
                nc.vector.tensor_scalar_mul(
                    o_sb, st[c]["p"], st[c]["rden"][0:CHUNK_GRAPHS, :])
                st[c]["o"] = o_sb

            def emit_store(c):
                # a dma_start's sem waits occupy the issuing SP sequencer and
                # would stall every x load queued behind it, so the store is
                # issued one iteration after the copy (top of the next
                # iteration), when the copy has long retired and the wait is
                # free.
                nc.sync.dma_start(
                    out=out_d[c * CHUNK_GRAPHS:(c + 1) * CHUNK_GRAPHS, :],
                    in_=st[c]["o"])
                # release references that are no longer needed
                del st[c]

            first = True
            for _rep in range(repeat):
                emit_load_xt(0, split=2)
                if first:
                    # late consts: only needed by score (w2) / softmax (mask)
                    if score_fmajor:
                        w2_sb = consts.tile([128, 2, 128], BF16, name="w2r_sb")
                    else:
                        w2_sb = consts.tile([128, 2], BF16, name="w2_sb")
                    nc.sync.dma_start(out=w2_sb, in_=w2_d)
                    mask_sb = consts.tile(
                        [TILE_NODES, CHUNK_GRAPHS, TILES_PER_CHUNK], BF16)
                    nc.sync.dma_start(out=mask_sb, in_=mask_d)
                    first = False
                if EN_XNAT:
                    emit_load_xnat(0)
                # the MLP tile stream runs ONE TILE AHEAD of the chunk
                # iteration: iteration c pulls tiles [4c+1 .. 4c+4], so the
                # next chunk's first h tile is emitted (and scheduled) before
                # the iteration boundary, hiding the boundary handoff.
                mg = mlp_tiles(4 * n_chunks)
                st[0]["s"] = s_both[:, 0, :]
                for _ in range(4):  # tile 0 during the pipeline fill
                    next(mg, None)
                for c in range(n_chunks + 3):
                    if EN_POOL and c >= 3:
                        emit_store(c - 3)
                    if c + 1 < n_chunks:
                        emit_load_xt(c + 1)
                        st[c + 1]["s"] = s_both[:, (c + 1) % 2, :]
                    sops = score_ops(c - 1) \
                        if EN_SCORE and 1 <= c <= n_chunks else None
                    pops = pool_ops(c - 2) \
                        if EN_POOL and 2 <= c < n_chunks + 2 else None
                    for i in range(TILES_PER_CHUNK):
                        next(mg, None)
                        if sops is not None and i < len(sops):
                            sops[i]()
                        # exp(c-1) is emitted at slot 8: its score matmuls
                        # (slots 0-3) clear the PE by ~1.5us, and ACT reaches
                        # the exp only after two tanh tiles (~2.1us), so the
                        # exp never stalls the ACT stream; the softmax chain
                        # still finishes mid-iteration, well before pooling
                        # needs E next iteration.
                        if sops is not None and i == 8:
                            emit_softmax(c - 1)
                        if pops is not None:
                            pops[i]()
                    if EN_POOL and 2 <= c < n_chunks + 2:
                        emit_copy_out(c - 2)
                    if EN_XNAT and c + 1 < n_chunks:
                        emit_load_xnat(c + 1)

    nc.compile()
    return nc


def _prep_inputs(x, W1, b1, W2, n_chunks_per_core, score_fmajor=False):
    """Host-side marshalling: bf16 cast, layouts, masks. Returns in_maps."""
    N, H = x.shape
    nodes_per_core = n_chunks_per_core * CHUNK_NODES

    xb = np.asarray(x).astype(NP_BF16)

    # natural layout: [core, chunk, p, t, h]
    x_nat = np.ascontiguousarray(
        xb.reshape(N_CORES, n_chunks_per_core, TILES_PER_CHUNK, TILE_NODES, H)
        .transpose(0, 1, 3, 2, 4))
    # transposed layout (fp8 e4m3): [core, kt, q, n_local]
    x_tr = np.ascontiguousarray(
        np.asarray(x).astype(NP_E4M3)
        .reshape(N_CORES, nodes_per_core, H).transpose(0, 2, 1)
        .reshape(N_CORES, 2, 128, nodes_per_core))

    # W1*64 as an e4m3 hi/lo pair: [p, hl, kt, mt, j]
    W1s = np.asarray(W1, np.float32) * W1_SCALE
    W1hi = W1s.astype(NP_E4M3)
    W1lo = (W1s - W1hi.astype(np.float32)).astype(NP_E4M3)
    w1_host = np.ascontiguousarray(
        np.stack([W1hi, W1lo])                   # [hl, hin, hout]
        .reshape(2, 2, 128, 2, 128).transpose(2, 0, 1, 3, 4))
    if score_fmajor:
        # replicated stationary operand: w2r[p, mt, m] = W2[mt*128+p]
        w2_host = np.ascontiguousarray(np.broadcast_to(
            np.asarray(W2).astype(NP_BF16).reshape(2, 128).T[:, :, None],
            (128, 2, 128)))
    else:
        w2_host = np.ascontiguousarray(
            np.asarray(W2).astype(NP_BF16).reshape(2, 128).T)   # [p, mt]
    b1_host = np.ascontiguousarray(
        np.asarray(b1).astype(np.float32).reshape(2, 128).T)  # [p, mt]

    # mask[p, g, t] = 1 iff node (t, p) of a chunk belongs to graph g
    p_idx = np.arange(TILE_NODES)
    t_idx = np.arange(TILES_PER_CHUNK)
    g_of_pt = 2 * t_idx[None, :] + p_idx[:, None] // GRAPH_NODES  # [p, t]
    mask_host = (g_of_pt[:, None, :] ==
                 np.arange(CHUNK_GRAPHS)[None, :, None]).astype(NP_BF16)

    in_maps = []
    for core in range(N_CORES):
        in_maps.append({
            "x_nat": x_nat[core],
            "x_tr": x_tr[core],
            "w1": w1_host,
            ("w2r" if score_fmajor else "w2"): w2_host,
            "b1": b1_host,
            "maskw": mask_host,
        })
    return in_maps


def _reference_numpy(x, batch, W1, b1, W2):
    """Fallback for non-uniform batch layouts: straight numpy."""
    x = np.asarray(x, dtype=np.float64)
    batch = np.asarray(batch).astype(np.int64)
    # the reference uses a fixed segment count (num_graphs = num_nodes/64),
    # not batch.max()+1 — keep trailing empty graphs as zero rows
    n_graphs = max(int(batch.max()) + 1, x.shape[0] // GRAPH_NODES)
    scores = np.tanh(x @ np.asarray(W1, np.float64) +
                     np.asarray(b1, np.float64)) @ np.asarray(W2, np.float64)
    scores = scores[:, 0]
    chunk_id = batch // CHUNK_GRAPHS
    n_chunks = int(chunk_id.max()) + 1
    m = np.full(n_chunks, -np.inf)
    np.maximum.at(m, chunk_id, scores)
    e = np.exp(scores - m[chunk_id])
    denom = np.zeros(n_chunks)
    np.add.at(denom, chunk_id, e)
    w = e / denom[chunk_id]
    out = np.zeros((n_graphs, x.shape[1]))
    np.add.at(out, batch, w[:, None] * x)
    return out.astype(np.float32)


SCORE_FMAJOR = False

def kernel(x, batch, W1, b1, W2, trace=False):
    x = np.asarray(x)
    batch = np.asarray(batch)
    N, H = x.shape
    n_graphs = int(batch[-1]) + 1

    # This kernel is specialized for the uniform sorted batch that the
    # reference generator produces (64 nodes per graph). Anything else
    # falls back to a host computation.
    expected = (np.arange(N, dtype=np.int64) * n_graphs) // N
    if (H != HIDDEN or N % (N_CORES * CHUNK_NODES) != 0
            or n_graphs % (N_CORES * CHUNK_GRAPHS) != 0
            or not np.array_equal(batch.astype(np.int64), expected)):
        return _reference_numpy(x, batch, W1, b1, W2)

    n_chunks_per_core = N // (N_CORES * CHUNK_NODES)

    key = (n_chunks_per_core, SCORE_FMAJOR)
    if key not in _NC_CACHE:
        _NC_CACHE[key] = build_nc(n_chunks_per_core,
                                  score_fmajor=SCORE_FMAJOR)
    nc = _NC_CACHE[key]

    in_maps = _prep_inputs(x, W1, b1, W2, n_chunks_per_core,
                           score_fmajor=SCORE_FMAJOR)
    try:
        res = run_bass_kernel_spmd(nc, in_maps, core_ids=list(range(N_CORES)),
                                   trace=trace)
    except ModuleNotFoundError:
        # NTFF trace hooks unavailable in this environment
        res = run_bass_kernel_spmd(nc, in_maps, core_ids=list(range(N_CORES)),
                                   trace=False)
    out = np.concatenate([r["out"] for r in res.results], axis=0)
    if trace:
        kernel.last_results = res
    return out.astype(np.float32)



# revision 75
# speedup vs baseline: 1.0183x; 1.0183x over previous
"""AttentionReadout kernel for Trainium2 (8 NeuronCores, data-parallel by chunk).

Reference computation (per full input):
    scores = (tanh(x @ W1 + b1) @ W2)[:, 0]          # [N]
    chunk_id = batch // 32                            # 32 graphs per chunk
    w = softmax of scores within each chunk           # [N]
    out = segment_sum(w[:, None] * x, batch)          # [4096, 256]

Shapes: x [262144, 256] f32, batch [262144] i64 (sorted, uniform: 64
nodes/graph), W1 [256,256], b1 [256], W2 [256,1].

Strategy (per core, 32768 nodes = 16 chunks of 2048 nodes):
  - host: ship x twice: natural layout in bf16 (pooling rhs; pooling
    needs ~bf16 precision to clear the 2e-2 gate) and transposed layout
    in fp8-e4m3 (MLP rhs; fp8 halves that DMA stream and enables
    DoubleRow matmuls). W1 is scaled by 64 (avoids the e4m3 subnormal
    region) and shipped as an e4m3 hi/lo pair so its quantization error
    cancels to ~0.1%; measured end-to-end rel err 8.3e-3 on the
    reference seed vs 2.5e-3 for all-bf16.
  - device, per chunk:
      hT = W1.T @ xT          (PE, fp8 DoubleRow: K=256 per matmul via
                               paired k-tiles; hi+lo accumulate in psum)
      th = tanh(hT/64 + b1)   (ACT, psum->sbuf bf16, scale=1/64)
      s[n] = th.T @ W2        (PE, tanh tile as stationary operand -> s in
                               node-partition layout [128,16] psum)
      e = exp(s)              (ACT; weights stay UNNORMALIZED)
      E[n, g] = e * mask      (DVE, mask precomputed on host)
      D = allreduce(rowsum e) (DVE free-axis reduce + GPSIMD all_reduce;
                               overlaps the pooling)
      out[g,:] = E.T @ x      (PE, accumulate 16 node-tiles per chunk)
      store (1/D) * out       (DVE tensor_scalar on the output copy)
  - softmax max-subtraction is skipped: scores = tanh(.)@W2 are bounded by
    sum|W2| <= 16, so exp() cannot overflow in f32 and e/sum(e) is
    mathematically identical to the max-shifted form.
  - scheduling: the MLP matmuls and tanh are emitted under
    tc.high_priority() (the tanh stream is the critical resource); the MLP
    psum rotates over THREE 2-bank buffers, enabled by packing the score
    and pooling accumulators of alternating chunks into one bank each;
    stores issue from SP one iteration after their producing copy so the
    SP sequencer never waits with x loads queued behind it.
"""

import contextlib

import numpy as np
import ml_dtypes

import concourse.bass as bass
import concourse.bacc as bacc
import concourse.tile as tile
import concourse.mybir as mybir
import concourse.bass_isa as bass_isa
from concourse.bass_utils import run_bass_kernel_spmd

BF16 = mybir.dt.bfloat16
F32 = mybir.dt.float32
FP8 = mybir.dt.float8e4
NP_BF16 = ml_dtypes.bfloat16
NP_E4M3 = ml_dtypes.float8_e4m3
W1_SCALE = 64.0  # shifts W1 (~±1/16) out of e4m3's subnormal range

N_CORES = 8
HIDDEN = 256
CHUNK_GRAPHS = 32
GRAPH_NODES = 64          # uniform: nodes per graph
TILE_NODES = 128          # nodes per node-tile (SBUF partition dim)
CHUNK_NODES = CHUNK_GRAPHS * GRAPH_NODES      # 2048
TILES_PER_CHUNK = CHUNK_NODES // TILE_NODES   # 16
BLOCKS_PER_CHUNK = 4                          # sub-blocks of 512 nodes
BLOCK_NODES = 512

_NC_CACHE = {}

HPSUM_BUFS = 3
# build-bisection switches (sim experiments only; all True for real runs)
EN_XNAT = True     # load x_nat
EN_POOL = True     # pooling matmuls + store
EN_SCORE = True    # score matmuls + softmax chain
EN_TANH = True     # tanh activations
POOL_FAKE_E = False  # probe: pool with mask (always ready) instead of E


def build_nc(n_chunks, repeat=1, out_name="out", salt=0, score_fmajor=False):
    """Build the per-core Bass program (identical across cores)."""
    nc = bacc.Bacc("TRN2", target_bir_lowering=False, debug=False,
                   enable_asserts=False)

    nodes = n_chunks * CHUNK_NODES
    # DRAM I/O (per-core shard)
    x_nat_d = nc.dram_tensor(
        "x_nat", [n_chunks, TILE_NODES, TILES_PER_CHUNK, HIDDEN], BF16,
        kind="ExternalInput").ap()
    x_tr_d = nc.dram_tensor(
        "x_tr", [2, 128, nodes], FP8, kind="ExternalInput").ap()
    # [p, hl, kt, mt, j]: e4m3 hi/lo pair of W1*64
    w1_d = nc.dram_tensor("w1", [128, 2, 2, 2, 128], FP8,
                          kind="ExternalInput").ap()
    if score_fmajor:
        w2_d = nc.dram_tensor("w2r", [128, 2, 128], BF16,
                              kind="ExternalInput").ap()
    else:
        w2_d = nc.dram_tensor("w2", [128, 2], BF16, kind="ExternalInput").ap()
    b1_d = nc.dram_tensor("b1", [128, 2], F32, kind="ExternalInput").ap()
    mask_d = nc.dram_tensor(
        "maskw", [TILE_NODES, CHUNK_GRAPHS, TILES_PER_CHUNK], BF16,
        kind="ExternalInput").ap()
    out_d = nc.dram_tensor(
        out_name, [n_chunks * CHUNK_GRAPHS, HIDDEN], F32,
        kind="ExternalOutput").ap()
    if salt:
        # dummy input whose shape varies per build variant: defeats
        # executable-cache dedup between otherwise identical HLO programs
        nc.dram_tensor("salt", [1, salt], F32, kind="ExternalInput")

    with tile.TileContext(nc) as tc:
        with (
            tc.tile_pool(name="consts", bufs=1) as consts,
            tc.tile_pool(name="xpool", bufs=6) as xpool,
            tc.tile_pool(name="xtpool", bufs=5) as xtpool,
            tc.tile_pool(name="thpool", bufs=10) as thpool,
            tc.tile_pool(name="epool", bufs=3) as epool,
            tc.tile_pool(name="opool", bufs=3) as opool,
            tc.tile_pool(name="swpool", bufs=2) as swpool,
            tc.tile_pool(name="hpsum", bufs=HPSUM_BUFS, space="PSUM") as hpsum,
            tc.tile_pool(name="spsum", bufs=1, space="PSUM") as spsum,
            tc.tile_pool(name="ppsum", bufs=1, space="PSUM") as ppsum,
        ):
            # The ACT function-table load (1.3us) starts at t~0 off a dummy
            # tanh whose source is a DVE-memset tile — no DMA dependency.
            warm_sb = consts.tile([128, 2], F32, name="act_warm")
            nc.vector.memset(warm_sb[:, 0:1], 0.0)
            nc.scalar.activation(warm_sb[:, 1:2], warm_sb[:, 0:1],
                                 mybir.ActivationFunctionType.Tanh)
            # DMA issue order is the fill critical path: the SP sequencer
            # spends ~565ns per dma_start, so the MLP inputs (w1, b1, then
            # xt(0) interleaved below) go first.
            w1_sb = consts.tile([128, 2, 2, 2, 128], FP8)
            nc.sync.dma_start(out=w1_sb, in_=w1_d)
            b1_sb = consts.tile([128, 2], F32)
            nc.sync.dma_start(out=b1_sb, in_=b1_d)

            # Persistent psum accumulators, two logical buffers packed into a
            # single bank each (the score tile is 64B/partition and the pool
            # tile 1KB/partition, but the tile-pool allocator rounds each buf
            # up to a full 2KB bank). Alternating chunks use the two halves;
            # a start=True matmul pending-zeroes its whole bank, so the
            # framework orders each chunk's first score/pool matmul after the
            # other half's last reader (exp/copy of chunk c-1) — about half
            # an iteration of slack. This frees two banks, buying the MLP
            # psum a third buffer (decouples the PE->ACT tanh ping-pong).
            # widened to the full bank: the tail borrows the dead space of half 0
            # as the LAST chunk's pooling accumulator, so pool(15) does not
            # serialize behind copy(14) on the shared p bank
            s_both = spsum.tile([128, 2, HIDDEN], F32, name="s_both")
            p_both = ppsum.tile([CHUNK_GRAPHS, 2, HIDDEN], F32, name="p_both")

            # Software pipeline, 3 chunks deep on the PE:
            #   iteration c emits, round-robin per node-tile:
            #     MLP matmuls of chunk c   (F=512 streams)
            #     score matmuls of c-1     (weight-load bound; hides under MLP)
            #     pooling matmuls of c-2   (F=256 streams)
            #   plus tanh(c) on ACT and the softmax chain (c-1) on
            #   ACT/GPSIMD/DVE, which runs while chunk c streams.
            st = {}  # per-chunk live tiles  (reassigned per repeat)

            def emit_load_xt(c, split=1):
                # xt issues at the top of the iteration: the HWDGE ring is
                # FIFO, and the transposed layout feeds the MLP immediately,
                # while the natural layout is only needed two pipeline
                # iterations later (pooling) and is queued at the bottom.
                # For chunk 0 the load is split so the first MLP block can
                # start after ~1/4 of the transfer (shorter pipeline fill).
                xt_sb = xtpool.tile([128, 2, CHUNK_NODES], FP8, tag="xt")
                step = CHUNK_NODES // split
                for s in range(split):
                    lo = s * step
                    nc.sync.dma_start(
                        out=xt_sb[:, :, lo:lo + step],
                        in_=x_tr_d[:, :, c * CHUNK_NODES + lo:
                                   c * CHUNK_NODES + lo + step]
                        .transpose([1, 0, 2]))
                st[c] = {"xt": xt_sb, "th": {}}

            def emit_load_xnat(c):
                x_sb = xpool.tile([TILE_NODES, TILES_PER_CHUNK, HIDDEN], BF16,
                                  tag="x")
                nc.sync.dma_start(out=x_sb, in_=x_nat_d[c])
                st[c]["x"] = x_sb

            def mlp_tiles(total_tiles):
                """Global MLP tile stream; yields after each DR matmul
                (4 per tile: 2 node-blocks x W1 hi/lo)."""
                for g in range(total_tiles):
                    c, r = divmod(g, 4)
                    bp, mt = divmod(r, 2)
                    xt_sb = st[c]["xt"]
                    h_ps = hpsum.tile([128, 2, BLOCK_NODES], F32, tag="h",
                                      name=f"h_ps{c}_{bp}_{mt}")
                    for bb in range(2):
                        nlo = (2 * bp + bb) * BLOCK_NODES
                        # DoubleRow: lhsT [128, kt=2, 128] pairs with
                        # rhs [128, kt=2, 512] -> full K=256 per matmul;
                        # hi then lo of W1*64 accumulate into psum.
                        for hl in range(2):
                            with tc.high_priority():
                                nc.tensor.matmul(
                                    h_ps[:, bb, :], w1_sb[:, hl, :, mt, :],
                                    xt_sb[:, :, nlo:nlo + BLOCK_NODES],
                                    start=(hl == 0), stop=(hl == 1),
                                    perf_mode=mybir.MatmulPerfMode.DoubleRow)
                            yield
                    th = thpool.tile([128, 2, BLOCK_NODES], BF16, tag="th",
                                     name=f"th{c}_{bp}_{mt}")
                    if EN_TANH:
                        with tc.high_priority():
                            nc.scalar.activation(
                                th, h_ps,
                                mybir.ActivationFunctionType.Tanh,
                                bias=b1_sb[:, mt:mt + 1],
                                scale=1.0 / W1_SCALE)
                    st[c]["th"][(bp, mt)] = th

            def score_ops(c):
                """8 slots; slot si emits the 4 accumulating F=1 matmuls for
                node-tiles 2si and 2si+1 (tanh as stationary operand).
                Compressed into the first half of the iteration so the exp
                for this chunk can issue mid-iteration, filling the ACT gap
                between tanh tiles and shortening the softmax chain."""
                ops = []
                # at the tail the scores gate the last exp but rank behind
                # pooling 107ns matmuls on the in-order PE; hoist them
                prio = (lambda: tc.high_priority()) if c == n_chunks - 1 \
                    else (lambda: contextlib.nullcontext())
                for si in range(TILES_PER_CHUNK // 4):
                    def op(si=si, c=c):
                        s_ps = st[c]["s"]
                        with prio():
                            for t in range(4 * si, 4 * si + 4):
                                b, tl = divmod(t, 4)
                                bp, bb = divmod(b, 2)
                                for mt in range(2):
                                    th = st[c]["th"][(bp, mt)]
                                    nc.tensor.matmul(
                                        s_ps[:, t:t + 1],
                                        th[:, bb, tl * 128:(tl + 1) * 128],
                                        w2_sb[:, mt:mt + 1],
                                        start=(mt == 0), stop=(mt == 1))
                    ops.append(op)
                return ops

            def emit_softmax(c, split=False):
                # Pool with UNNORMALIZED weights E = exp(s)*mask; the 1/D
                # factor is applied per-graph on the output copy instead.
                # This takes the accumulator read off ACT and the reciprocal
                # off the exp->pool critical chain (it overlaps pooling).
                # For the LAST chunk the exp/E are split in half so its
                # pooling can overlap the final tanh tiles (shorter tail);
                # in steady state one exp per chunk keeps ACT lean.
                e_sb = epool.tile([128, TILES_PER_CHUNK], BF16, tag="e")
                e_full = epool.tile(
                    [TILE_NODES, CHUNK_GRAPHS, TILES_PER_CHUNK], BF16,
                    tag="efull")
                halves = ((0, 8), (8, 16)) if split else ((0, 16),)
                # At the tail there are no more MLP tiles to hide behind, so
                # the last chunks' exp/E run at high priority (their score
                # matmuls are long done); in steady state the exp slots into
                # the boundary gap on its default priority.
                prio = tc.high_priority() if c >= n_chunks - 2 \
                    else contextlib.nullcontext()
                with prio:
                    for lo, hi in halves:
                        nc.scalar.activation(
                            e_sb[:, lo:hi], st[c]["s"][:, lo:hi],
                            mybir.ActivationFunctionType.Exp)
                        e_bc = e_sb[:, lo:hi].unsqueeze(1).broadcast_to(
                            [TILE_NODES, CHUNK_GRAPHS, hi - lo])
                        nc.vector.tensor_mul(
                            e_full[:, :, lo:hi], e_bc, mask_sb[:, :, lo:hi])
                acc = epool.tile([128, 1], F32, tag="acc")
                nc.vector.tensor_reduce(
                    acc, e_sb, mybir.AxisListType.X, mybir.AluOpType.add)
                dsum = epool.tile([128, 1], F32, tag="dsum")
                nc.gpsimd.partition_all_reduce(
                    dsum, acc, 128, bass_isa.ReduceOp.add)
                rden = epool.tile([128, 1], F32, tag="rden")
                nc.vector.reciprocal(rden, dsum)
                st[c]["rden"] = rden
                st[c]["E"] = e_full
                if c == n_chunks - 1:
                    st[c]["p"] = s_both[0:CHUNK_GRAPHS, (c + 1) % 2, :]
                else:
                    st[c]["p"] = p_both[:, c % 2, :]

            def pool_ops(c):
                ops = []
                for t in range(TILES_PER_CHUNK):
                    def op(t=t, c=c):
                        e_op = mask_sb if POOL_FAKE_E else st[c]["E"]
                        nc.tensor.matmul(
                            st[c]["p"], e_op[:, :, t], st[c]["x"][:, t, :],
                            start=(t == 0), stop=(t == TILES_PER_CHUNK - 1))
                    ops.append(op)
                return ops

            def emit_copy_out(c):
                o_sb = opool.tile([CHUNK_GRAPHS, HIDDEN], F32, tag="o")
                # normalize while copying: all partitions of rden hold the
                # chunk's all-reduced 1/D, so rows 0-31 see the same scalar
                nc.vector.tensor_scalar_mul(
                    o_sb, st[c]["p"], st[c]["rden"][0:CHUNK_GRAPHS, :])
                st[c]["o"] = o_sb

            def emit_store(c):
                # a dma_start's sem waits occupy the issuing SP sequencer and
                # would stall every x load queued behind it, so the store is
                # issued one iteration after the copy (top of the next
                # iteration), when the copy has long retired and the wait is
                # free.
                nc.sync.dma_start(
                    out=out_d[c * CHUNK_GRAPHS:(c + 1) * CHUNK_GRAPHS, :],
                    in_=st[c]["o"])
                # release references that are no longer needed
                del st[c]

            first = True
            for _rep in range(repeat):
                emit_load_xt(0, split=2)
                if first:
                    # late consts: only needed by score (w2) / softmax (mask)
                    if score_fmajor:
                        w2_sb = consts.tile([128, 2, 128], BF16, name="w2r_sb")
                    else:
                        w2_sb = consts.tile([128, 2], BF16, name="w2_sb")
                    nc.sync.dma_start(out=w2_sb, in_=w2_d)
                    mask_sb = consts.tile(
                        [TILE_NODES, CHUNK_GRAPHS, TILES_PER_CHUNK], BF16)
                    nc.sync.dma_start(out=mask_sb, in_=mask_d)
                    first = False
                if EN_XNAT:
                    emit_load_xnat(0)
                # the MLP tile stream runs ONE TILE AHEAD of the chunk
                # iteration: iteration c pulls tiles [4c+1 .. 4c+4], so the
                # next chunk's first h tile is emitted (and scheduled) before
                # the iteration boundary, hiding the boundary handoff.
                mg = mlp_tiles(4 * n_chunks)
                st[0]["s"] = s_both[:, 0, 0:TILES_PER_CHUNK]
                for _ in range(4):  # tile 0 during the pipeline fill
                    next(mg, None)
                for c in range(n_chunks + 3):
                    if EN_POOL and c >= 3:
                        emit_store(c - 3)
                    if c + 1 < n_chunks:
                        emit_load_xt(c + 1)
                        st[c + 1]["s"] = s_both[:, (c + 1) % 2, 0:TILES_PER_CHUNK]
                    sops = score_ops(c - 1) \
                        if EN_SCORE and 1 <= c <= n_chunks else None
                    pops = pool_ops(c - 2) \
                        if EN_POOL and 2 <= c < n_chunks + 2 else None
                    for i in range(TILES_PER_CHUNK):
                        next(mg, None)
                        if sops is not None and i < len(sops):
                            sops[i]()
                        # exp(c-1) is emitted at slot 8: its score matmuls
                        # (slots 0-3) clear the PE by ~1.5us, and ACT reaches
                        # the exp only after two tanh tiles (~2.1us), so the
                        # exp never stalls the ACT stream; the softmax chain
                        # still finishes mid-iteration, well before pooling
                        # needs E next iteration.
                        if sops is not None and i == 8:
                            emit_softmax(c - 1, split=(c - 1 == n_chunks - 1))
                        if pops is not None:
                            pops[i]()
                    if EN_POOL and 2 <= c < n_chunks + 2:
                        emit_copy_out(c - 2)
                    if EN_XNAT and c + 1 < n_chunks:
                        emit_load_xnat(c + 1)

    nc.compile()
    return nc


def _prep_inputs(x, W1, b1, W2, n_chunks_per_core, score_fmajor=False):
    """Host-side marshalling: bf16 cast, layouts, masks. Returns in_maps."""
    N, H = x.shape
    nodes_per_core = n_chunks_per_core * CHUNK_NODES

    xb = np.asarray(x).astype(NP_BF16)

    # natural layout: [core, chunk, p, t, h]
    x_nat = np.ascontiguousarray(
        xb.reshape(N_CORES, n_chunks_per_core, TILES_PER_CHUNK, TILE_NODES, H)
        .transpose(0, 1, 3, 2, 4))
    # transposed layout (fp8 e4m3): [core, kt, q, n_local]
    x_tr = np.ascontiguousarray(
        np.asarray(x).astype(NP_E4M3)
        .reshape(N_CORES, nodes_per_core, H).transpose(0, 2, 1)
        .reshape(N_CORES, 2, 128, nodes_per_core))

    # W1*64 as an e4m3 hi/lo pair: [p, hl, kt, mt, j]
    W1s = np.asarray(W1, np.float32) * W1_SCALE
    W1hi = W1s.astype(NP_E4M3)
    W1lo = (W1s - W1hi.astype(np.float32)).astype(NP_E4M3)
    w1_host = np.ascontiguousarray(
        np.stack([W1hi, W1lo])                   # [hl, hin, hout]
        .reshape(2, 2, 128, 2, 128).transpose(2, 0, 1, 3, 4))
    if score_fmajor:
        # replicated stationary operand: w2r[p, mt, m] = W2[mt*128+p]
        w2_host = np.ascontiguousarray(np.broadcast_to(
            np.asarray(W2).astype(NP_BF16).reshape(2, 128).T[:, :, None],
            (128, 2, 128)))
    else:
        w2_host = np.ascontiguousarray(
            np.asarray(W2).astype(NP_BF16).reshape(2, 128).T)   # [p, mt]
    b1_host = np.ascontiguousarray(
        np.asarray(b1).astype(np.float32).reshape(2, 128).T)  # [p, mt]

    # mask[p, g, t] = 1 iff node (t, p) of a chunk belongs to graph g
    p_idx = np.arange(TILE_NODES)
    t_idx = np.arange(TILES_PER_CHUNK)
    g_of_pt = 2 * t_idx[None, :] + p_idx[:, None] // GRAPH_NODES  # [p, t]
    mask_host = (g_of_pt[:, None, :] ==
                 np.arange(CHUNK_GRAPHS)[None, :, None]).astype(NP_BF16)

    in_maps = []
    for core in range(N_CORES):
        in_maps.append({
            "x_nat": x_nat[core],
            "x_tr": x_tr[core],
            "w1": w1_host,
            ("w2r" if score_fmajor else "w2"): w2_host,
            "b1": b1_host,
            "maskw": mask_host,
        })
    return in_maps


def _reference_numpy(x, batch, W1, b1, W2):
    """Fallback for non-uniform batch layouts: straight numpy."""
    x = np.asarray(x, dtype=np.float64)
    batch = np.asarray(batch).astype(np.int64)
    # the reference uses a fixed segment count (num_graphs = num_nodes/64),
    # not batch.max()+1 — keep trailing empty graphs as zero rows
    n_graphs = max(int(batch.max()) + 1, x.shape[0] // GRAPH_NODES)
    scores = np.tanh(x @ np.asarray(W1, np.float64) +
                     np.asarray(b1, np.float64)) @ np.asarray(W2, np.float64)
    scores = scores[:, 0]
    chunk_id = batch // CHUNK_GRAPHS
    n_chunks = int(chunk_id.max()) + 1
    m = np.full(n_chunks, -np.inf)
    np.maximum.at(m, chunk_id, scores)
    e = np.exp(scores - m[chunk_id])
    denom = np.zeros(n_chunks)
    np.add.at(denom, chunk_id, e)
    w = e / denom[chunk_id]
    out = np.zeros((n_graphs, x.shape[1]))
    np.add.at(out, batch, w[:, None] * x)
    return out.astype(np.float32)


SCORE_FMAJOR = False

def kernel(x, batch, W1, b1, W2, trace=False):
    x = np.asarray(x)
    batch = np.asarray(batch)
    N, H = x.shape
    n_graphs = int(batch[-1]) + 1

    # This kernel is specialized for the uniform sorted batch that the
    # reference generator produces (64 nodes per graph). Anything else
    # falls back to a host computation.
    expected = (np.arange(N, dtype=np.int64) * n_graphs) // N
    if (H != HIDDEN or N % (N_CORES * CHUNK_NODES) != 0
            or n_graphs % (N_CORES * CHUNK_GRAPHS) != 0
            or not np.array_equal(batch.astype(np.int64), expected)):
        return _reference_numpy(x, batch, W1, b1, W2)

    n_chunks_per_core = N // (N_CORES * CHUNK_NODES)

    key = (n_chunks_per_core, SCORE_FMAJOR)
    if key not in _NC_CACHE:
        _NC_CACHE[key] = build_nc(n_chunks_per_core,
                                  score_fmajor=SCORE_FMAJOR)
    nc = _NC_CACHE[key]

    in_maps = _prep_inputs(x, W1, b1, W2, n_chunks_per_core,
                           score_fmajor=SCORE_FMAJOR)
    try:
        res = run_bass_kernel_spmd(nc, in_maps, core_ids=list(range(N_CORES)),
                                   trace=trace)
    except ModuleNotFoundError:
        # NTFF trace hooks unavailable in this environment
        res = run_bass_kernel_spmd(nc, in_maps, core_ids=list(range(N_CORES)),
                                   trace=False)
    out = np.concatenate([r["out"] for r in res.results], axis=0)
    if trace:
        kernel.last_results = res
    return out.astype(np.float32)

